# revision 6
# baseline (speedup 1.0000x reference)
"""MoE ExpertRouter kernel for Trainium2 (8 NeuronCores, SPMD).

Reference computation (per token t of T=16384, H=2048, E=8, K=2):
    logits = x_flat @ gate_w.T                  # (T, E)
    probs  = softmax(logits)
    w, i   = top_k(probs, 2); w /= w.sum(-1)    # == softmax over top-2 logits
    aux    = var(one_hot(i).sum(1).mean(0), ddof=1) * E

Sharding: data-parallel over the flattened token dim, 2048 tokens/core.
gate_w is tiny and replicated (host pre-transposes it to [H, E]).
Per core the device kernel:
  - streams x [2048, 2048] f32 in 1 MiB tiles,
  - PE-transposes 128x128 blocks (fp32 has no DMA-transpose path),
  - matmuls xT chunks against gate_w.T accumulating logits in PSUM,
  - DVE max/max_index gives the top-8 sorted values + indices per token,
  - top-2 weights: w1 = 1/(1+exp(v2-v1)), w2 = exp(v2-v1)*w1.
The per-expert counts / aux loss are reduced on host from the gathered
indices (the all-reduce equivalent over 8 scalar values per core).
"""

import numpy as np

import concourse.bass as bass
import concourse.mybir as mybir
from concourse import bacc
from concourse.bass_utils import run_bass_kernel_spmd
from concourse.tile import TileContext

B, S, H, E, K = 4, 4096, 2048, 8, 2
T = B * S                # 16384 tokens total
NCORES = 8
TPC = T // NCORES        # 2048 tokens per core
P = 128                  # partitions
G = TPC // P             # 16 token groups (of 128 tokens) per core
HCHUNKS = H // P         # 16 contraction chunks of 128


def build_program(repeat=1):
    nc = bacc.Bacc(None, target_bir_lowering=False)

    xs = nc.dram_tensor("xs", [TPC, H], mybir.dt.float32, kind="ExternalInput")
    wt = nc.dram_tensor("wt", [H, E], mybir.dt.float32, kind="ExternalInput")
    ident = nc.dram_tensor("ident", [P, P], mybir.dt.float32, kind="ExternalInput")
    ow = nc.dram_tensor("ow", [P, G * K], mybir.dt.float32, kind="ExternalOutput")
    oi = nc.dram_tensor("oi", [P, G * K], mybir.dt.uint32, kind="ExternalOutput")

    with TileContext(nc) as tc:
        with (
            tc.tile_pool(name="const", bufs=1) as const_pool,
            tc.tile_pool(name="xin", bufs=3) as xin_pool,
            tc.tile_pool(name="xt", bufs=4) as xt_pool,
            tc.tile_pool(name="tp_psum", bufs=2, space="PSUM") as tp_psum_pool,
            tc.tile_pool(name="lg_psum", bufs=2, space="PSUM") as lg_psum_pool,
            tc.tile_pool(name="lg_sb", bufs=2) as lg_sb_pool,
            tc.tile_pool(name="stats", bufs=1) as stats_pool,
        ):
            ident_sb = const_pool.tile([P, P], mybir.dt.float32)
            nc.sync.dma_start(out=ident_sb, in_=ident[:])
            # wt_sb[p, 8*c+e] = wt[128*c+p, e]  (H-chunk c on cols 8c:8c+8)
            wt_sb = const_pool.tile([P, HCHUNKS * E], mybir.dt.float32)
            nc.sync.dma_start(
                out=wt_sb.rearrange("p (c e) -> p c e", e=E),
                in_=wt[:].rearrange("(c p) e -> p c e", p=P),
            )

            vmax_all = stats_pool.tile([P, G * E], mybir.dt.float32)
            vidx_all = stats_pool.tile([P, G * E], mybir.dt.uint32)

            lg_bank = None
            for g in range(G * repeat):
                g = g % G
                x_tile = xin_pool.tile([P, H], mybir.dt.float32)
                nc.sync.dma_start(out=x_tile, in_=xs[g * P : (g + 1) * P, :])

                if g % 4 == 0:
                    lg_bank = lg_psum_pool.tile([P, 4 * E], mybir.dt.float32)
                lg_slice = lg_bank[:, (g % 4) * E : (g % 4 + 1) * E]

                for jj in range(4):  # 4 super-chunks of 512 along H
                    tp = tp_psum_pool.tile([P, 512], mybir.dt.float32)
                    for k in range(4):
                        j = 4 * jj + k
                        nc.tensor.transpose(
                            tp[:, k * P : (k + 1) * P],
                            x_tile[:, j * P : (j + 1) * P],
                            ident_sb,
                        )
                    xt_sb = xt_pool.tile([P, 512], mybir.dt.float32)
                    # alternate copy engine to split PSUM->SBUF traffic
                    if jj % 2 == 0:
                        nc.scalar.copy(out=xt_sb, in_=tp)
                    else:
                        nc.vector.tensor_copy(out=xt_sb, in_=tp)
                    for k in range(4):
                        j = 4 * jj + k
                        nc.tensor.matmul(
                            lg_slice,
                            lhsT=xt_sb[:, k * P : (k + 1) * P],
                            rhs=wt_sb[:, j * E : (j + 1) * E],
                            start=(j == 0),
                            stop=(j == HCHUNKS - 1),
                        )

                if g % 4 == 3:
                    lgs = lg_sb_pool.tile([P, 4 * E], mybir.dt.float32)
                    nc.vector.tensor_copy(out=lgs, in_=lg_bank)
                    for k in range(4):
                        gg = g - 3 + k
                        nc.vector.max(
                            out=vmax_all[:, gg * E : (gg + 1) * E],
                            in_=lgs[:, k * E : (k + 1) * E],
                        )
                        nc.vector.max_index(
                            out=vidx_all[:, gg * E : (gg + 1) * E],
                            in_max=vmax_all[:, gg * E : (gg + 1) * E],
                            in_values=lgs[:, k * E : (k + 1) * E],
                        )

            # ---- epilogue: weights from top-2 logits --------------------
            # v1 = vmax_all[:, 0::E], v2 = vmax_all[:, 1::E]   (both [P, G])
            d = stats_pool.tile([P, G], mybir.dt.float32)
            e2 = stats_pool.tile([P, G], mybir.dt.float32)
            s = stats_pool.tile([P, G], mybir.dt.float32)
            ow_sb = stats_pool.tile([P, G * K], mybir.dt.float32)
            oi_sb = stats_pool.tile([P, G * K], mybir.dt.uint32)

            nc.vector.tensor_sub(d, vmax_all[:, 1 :: E], vmax_all[:, 0 :: E])
            nc.scalar.activation(e2, d, mybir.ActivationFunctionType.Exp)
            nc.vector.tensor_scalar_add(s, e2, 1.0)
            nc.vector.reciprocal(ow_sb[:, 0::K], s)
            nc.vector.tensor_mul(ow_sb[:, 1::K], e2, ow_sb[:, 0::K])
            nc.vector.tensor_copy(
                out=oi_sb.rearrange("p (g k) -> p g k", k=K),
                in_=vidx_all.rearrange("p (g e) -> p g e", e=E)[:, :, 0:K],
            )

            nc.sync.dma_start(out=ow[:], in_=ow_sb)
            nc.sync.dma_start(out=oi[:], in_=oi_sb)

    nc.compile()
    return nc


_NC_CACHE = None


def _get_program():
    global _NC_CACHE
    if _NC_CACHE is None:
        _NC_CACHE = build_program()
    return _NC_CACHE


def kernel(x, gate_w):
    x = np.asarray(x, dtype=np.float32)
    gate_w = np.asarray(gate_w, dtype=np.float32)

    xs = np.ascontiguousarray(x.reshape(T, H))
    wt = np.ascontiguousarray(gate_w.T)          # [H, E]
    ident = np.eye(P, dtype=np.float32)

    nc = _get_program()
    in_maps = [
        {"xs": xs[c * TPC : (c + 1) * TPC], "wt": wt, "ident": ident}
        for c in range(NCORES)
    ]
    res = run_bass_kernel_spmd(nc, in_maps, core_ids=list(range(NCORES)))

    ow = np.stack([res.results[c]["ow"] for c in range(NCORES)])  # [8,128,G*K]
    oi = np.stack([res.results[c]["oi"] for c in range(NCORES)])

    # [ncore, p, g, k] -> token (c*TPC + g*P + p)
    weights = (
        ow.reshape(NCORES, P, G, K).transpose(0, 2, 1, 3).reshape(T, K)
    ).astype(np.float32)
    indices = (
        oi.reshape(NCORES, P, G, K).transpose(0, 2, 1, 3).reshape(T, K)
    ).astype(np.int32)

    # aux loss from gathered indices (counts are exact integers)
    counts = np.bincount(indices.ravel(), minlength=E).astype(np.float32)
    mean_per_expert = counts / np.float32(T)
    m = np.float32(mean_per_expert.mean())
    var = np.float32(((mean_per_expert - m) ** 2).sum() / np.float32(E - 1))
    aux_loss = np.float32(var * E)

    return weights, indices, aux_loss
